# revision 2
# baseline (speedup 1.0000x reference)
"""Multi-head contextual biased attention on 8 Trainium2 NeuronCores.

Sharding: data-parallel over batch (B=2) x tensor-parallel over heads
(16 heads -> 4 per core). Each core computes Q/K/V projections for its
4 heads, streaming-softmax attention with the periodic ALiBi-style bias
applied as a precomputed multiplicative table (exp(bias) folded in after
exp(scores)), and the output projection. The host sums the 4 partial
output projections per batch element and adds the bias bo.

Device layout notes:
  - scores are computed transposed (S^T[j, i], context j on partitions) so
    the P@V contraction can run with V as the stationary operand; a ones
    column appended to V yields softmax denominators in the same matmul.
  - heads are processed in pairs (even head on partitions 0-63, odd head
    on 64-127); their score matmuls have a 64-deep contraction, so they
    lower to 64x128 row-tiles (0,0)/(64,0) of the PE array and execute
    concurrently when interleaved.  The i range is processed in halves so
    two score tiles [128,1024] plus two PV accumulators [65,1024] fit the
    8 PSUM banks exactly.
  - exp(bias*head_scale) depends only on (j - i), so it is stored as one
    skewed per-partition sequence eb[p, t] = g(p + t - T0) and addressed
    per tile with a step -1 access pattern; the multiply runs on the DVE
    for the even head and the Pool engine for the odd head so the ACT
    engine only runs the exp stream.
"""

import numpy as np
import ml_dtypes
from contextlib import ExitStack

import concourse.bass as bass
import concourse.tile as tile
from concourse import bacc, mybir
from concourse.bass_utils import run_bass_kernel_spmd

bf16 = ml_dtypes.bfloat16
F32 = mybir.dt.float32
BF16 = mybir.dt.bfloat16
Exp = mybir.ActivationFunctionType.Exp

B, T, D = 2, 2048, 1024
NH, DH = 16, 64          # global heads, head dim
HL = 4                   # heads per core
KC = D // 128            # contraction chunks
PERIOD = 30
T0 = 2049                # odd skew origin (odd => step -1 APs stay 4B-aligned)
EBL = 3972               # skew table length
HT = 1024                # i-half width


def _build_kernel(ctx, tc, y_d, xt_d, ct_d, wq_d, wk_d, wv_d, wo_d, eb_d):
    nc = tc.nc

    const = ctx.enter_context(tc.tile_pool(name="const", bufs=1))
    data = ctx.enter_context(tc.tile_pool(name="data", bufs=1))

    # DMA issue order matters: queues drain in issue order, so the q-path
    # inputs (wq, x) go first, then the k path, then pair-0's bias tables
    # (needed ~2us after the first scores land) and the late-use loads.
    wq_sb = const.tile([128, KC, 256], BF16)
    nc.sync.dma_start(wq_sb[:], wq_d[:])
    xt_sb = data.tile([128, KC, T], BF16)
    for k in range(KC):
        nc.sync.dma_start(xt_sb[:, k, :], xt_d[:, k, :])
    wk_sb = const.tile([128, KC, 256], BF16)
    nc.sync.dma_start(wk_sb[:], wk_d[:])
    ct_sb = data.tile([128, KC, T], BF16)
    for k in range(KC):
        nc.sync.dma_start(ct_sb[:, k, :], ct_d[:, k, :])
    wv_sb = const.tile([128, KC, 256], BF16)
    nc.sync.dma_start(wv_sb[:], wv_d[:])
    eb_sb = const.tile([128, HL, EBL], BF16)
    nc.sync.dma_start(eb_sb[:, 0:2, :], eb_d[:, 0:2, :])
    nc.sync.dma_start(eb_sb[:, 2:4, :], eb_d[:, 2:4, :])
    wo_sb = const.tile([128, 2, D], BF16)
    nc.sync.dma_start(wo_sb[:], wo_d[:])

    qT_sb = data.tile([128, 2, T], BF16)
    kT_sb = data.tile([128, 2, T], BF16)
    v_sb = data.tile([128, 16, HL, 65], BF16)
    o2_sb = data.tile([128, 2, T], BF16)
    nc.vector.memset(v_sb[:, :, :, 64:65], 1.0)

    # ---- Phase 1: projections ----
    with tc.tile_pool(name="pps", bufs=4, space="PSUM") as pps:
        def proj_qk(dst, w_sb, src_sb, m, nm):
            for it in range(4):
                ps = pps.tile([128, 512], F32, tag="mm", name=f"{nm}_{it}_{m}")
                for k in range(KC):
                    nc.tensor.matmul(ps[:], lhsT=w_sb[:, k, m * 128:(m + 1) * 128],
                                     rhs=src_sb[:, k, it * 512:(it + 1) * 512],
                                     start=(k == 0), stop=(k == KC - 1))
                nc.vector.tensor_copy(dst[:, m, it * 512:(it + 1) * 512], ps[:])

        proj_qk(qT_sb, wq_sb, xt_sb, 0, "qps")   # scale folded into wq on host
        proj_qk(kT_sb, wk_sb, ct_sb, 0, "kps")
        # v[j, d] in per-head stationary layout
        for jt in range(16):
            ps = pps.tile([128, 512], F32, tag="mm", name=f"vps_{jt}")
            for k in range(KC):
                nc.tensor.matmul(ps[:, 0:256], lhsT=ct_sb[:, k, jt * 128:(jt + 1) * 128],
                                 rhs=wv_sb[:, k, :], start=(k == 0), stop=(k == KC - 1))
            nc.vector.tensor_copy(
                v_sb[:, jt, :, 0:64],
                ps[:, 0:256].rearrange("p (h d) -> p h d", h=HL))
        proj_qk(qT_sb, wq_sb, xt_sb, 1, "qps")
        proj_qk(kT_sb, wk_sb, ct_sb, 1, "kps")

    # ---- Phase 2: attention, head pairs x i-halves ----
    with tc.tile_pool(name="spp", bufs=1, space="PSUM") as spp, \
         tc.tile_pool(name="pvp", bufs=1, space="PSUM") as pvp, \
         tc.tile_pool(name="ptp", bufs=2) as ptp, \
         tc.tile_pool(name="nrm", bufs=2) as nrm:
        for m in range(2):
            hA, hB = 2 * m, 2 * m + 1
            for half in range(2):
                i0 = half * HT
                sfx = f"{m}_{half}"
                pvA = pvp.tile([65, HT], F32, tag="pvA", name=f"pvA_{sfx}")
                pvB = pvp.tile([65, HT], F32, tag="pvB", name=f"pvB_{sfx}")

                def emit_pv(jt, ptA, ptB):
                    for it2 in range(2):
                        osl = slice(it2 * 512, (it2 + 1) * 512)
                        nc.tensor.matmul(pvA[:, osl], lhsT=v_sb[:, jt, hA, :],
                                         rhs=ptA[:, osl],
                                         start=(jt == 0), stop=(jt == 15))
                    for it2 in range(2):
                        osl = slice(it2 * 512, (it2 + 1) * 512)
                        nc.tensor.matmul(pvB[:, osl], lhsT=v_sb[:, jt, hB, :],
                                         rhs=ptB[:, osl],
                                         start=(jt == 0), stop=(jt == 15))

                prevA = prevB = None
                for jt in range(16):
                    jsl = slice(jt * 128, (jt + 1) * 128)
                    spA = spp.tile([128, HT], F32, tag="sA", name=f"sA_{sfx}_{jt}")
                    spB = spp.tile([128, HT], F32, tag="sB", name=f"sB_{sfx}_{jt}")
                    # interleave the pair's matmuls: A hits row-tile (0,0),
                    # B hits (64,0) -> they execute concurrently in the array
                    for it2 in range(2):
                        osl = slice(it2 * 512, (it2 + 1) * 512)
                        isl = slice(i0 + it2 * 512, i0 + it2 * 512 + 512)
                        nc.tensor.matmul(spA[:, osl], lhsT=kT_sb[0:64, m, jsl],
                                         rhs=qT_sb[0:64, m, isl],
                                         start=True, stop=True,
                                         tile_position=(0, 0))
                        nc.tensor.matmul(spB[:, osl], lhsT=kT_sb[64:128, m, jsl],
                                         rhs=qT_sb[64:128, m, isl],
                                         start=True, stop=True,
                                         tile_position=(64, 0))
                    ptA = ptp.tile([128, HT], BF16, tag="pA", name=f"pA_{sfx}_{jt}")
                    ptB = ptp.tile([128, HT], BF16, tag="pB", name=f"pB_{sfx}_{jt}")
                    nc.scalar.activation(ptA[:], spA[:], Exp)
                    nc.scalar.activation(ptB[:], spB[:], Exp)
                    idx = T0 + jt * 128 - i0
                    nc.vector.tensor_mul(ptA[:], ptA[:],
                                         eb_sb[:, hA, idx:idx - HT:-1])
                    nc.gpsimd.tensor_mul(ptB[:], ptB[:],
                                         eb_sb[:, hB, idx:idx - HT:-1])
                    if prevA is not None:
                        emit_pv(jt - 1, prevA, prevB)
                    prevA, prevB = ptA, ptB
                emit_pv(15, prevA, prevB)

                # fast psum release, then the (slow) normalization chain off
                # the critical path.
                pvfA = nrm.tile([65, HT], F32, tag="pvfA", name=f"pvfA_{sfx}")
                pvfB = nrm.tile([65, HT], F32, tag="pvfB", name=f"pvfB_{sfx}")
                nc.vector.tensor_copy(pvfA[:], pvA[:])
                nc.vector.tensor_copy(pvfB[:], pvB[:])
                for hb, pvf, h in ((0, pvfA, hA), (1, pvfB, hB)):
                    nsfx = f"{sfx}_{hb}"
                    rsq = nrm.tile([128, 8], F32, tag=f"rsq{hb}", name=f"rsq_{nsfx}")
                    nc.sync.dma_start(rsq[:], pvf[64:65, :])
                    rsr = nrm.tile([128, 8], F32, tag=f"rsr{hb}", name=f"rsr_{nsfx}")
                    nc.vector.reciprocal(rsr[:], rsq[:])
                    rsf = nrm.tile([1, HT], F32, tag=f"rsf{hb}", name=f"rsf_{nsfx}")
                    nc.sync.dma_start(rsf[:], rsr[:])
                    rsb = nrm.tile([64, HT], F32, tag=f"rsb{hb}", name=f"rsb_{nsfx}")
                    nc.gpsimd.partition_broadcast(rsb[:], rsf[:], channels=64)
                    # normalized heads land in o2 [128, 2, T]: even head ->
                    # partitions 0..63 directly; odd head -> staging + DMA.
                    if hb == 0:
                        nc.vector.tensor_mul(o2_sb[0:64, m, i0:i0 + HT],
                                             pvf[0:64, :], rsb[:])
                    else:
                        otmp = nrm.tile([64, HT], BF16, tag="otmp", name=f"otmp_{nsfx}")
                        nc.vector.tensor_mul(otmp[:], pvf[0:64, :], rsb[:])
                        nc.sync.dma_start(o2_sb[64:128, m, i0:i0 + HT], otmp[:])

    # ---- Phase 3: output projection (partial; host sums across head-groups) ----
    with tc.tile_pool(name="yps", bufs=3, space="PSUM") as yps, \
         tc.tile_pool(name="yo", bufs=8) as yo:
        for ic in range(16):
            for mt in range(2):
                ps = yps.tile([128, 512], F32, tag="y", name=f"yps_{ic}_{mt}")
                for m in range(2):
                    nc.tensor.matmul(ps[:], lhsT=o2_sb[:, m, ic * 128:(ic + 1) * 128],
                                     rhs=wo_sb[:, m, mt * 512:(mt + 1) * 512],
                                     start=(m == 0), stop=(m == 1))
                yt = yo.tile([128, 512], BF16, tag="yt", name=f"yt_{ic}_{mt}")
                # alternate cast engine so neither DVE nor ACT serializes the tail
                if mt == 0:
                    nc.vector.tensor_copy(yt[:], ps[:])
                else:
                    nc.scalar.copy(yt[:], ps[:])
                nc.sync.dma_start(y_d[ic * 128:(ic + 1) * 128, mt * 512:(mt + 1) * 512],
                                  yt[:])


_NC = None


def build_nc():
    global _NC
    if _NC is not None:
        return _NC
    nc = bacc.Bacc("TRN2", target_bir_lowering=False, debug=False, num_devices=8)
    xt_d = nc.dram_tensor("xt", [128, KC, T], BF16, kind="ExternalInput").ap()
    ct_d = nc.dram_tensor("ct", [128, KC, T], BF16, kind="ExternalInput").ap()
    wq_d = nc.dram_tensor("wq", [128, KC, 256], BF16, kind="ExternalInput").ap()
    wk_d = nc.dram_tensor("wk", [128, KC, 256], BF16, kind="ExternalInput").ap()
    wv_d = nc.dram_tensor("wv", [128, KC, 256], BF16, kind="ExternalInput").ap()
    wo_d = nc.dram_tensor("wo", [128, 2, D], BF16, kind="ExternalInput").ap()
    eb_d = nc.dram_tensor("eb", [128, HL, EBL], BF16, kind="ExternalInput").ap()
    y_d = nc.dram_tensor("y", [T, D], BF16, kind="ExternalOutput").ap()

    with tile.TileContext(nc) as tc, ExitStack() as ctx:
        _build_kernel(ctx, tc, y_d, xt_d, ct_d, wq_d, wk_d, wv_d, wo_d, eb_d)
    nc.compile()
    _NC = nc
    return nc


def _to_chunked(mat_t, cols):
    """[D, cols] -> [128, KC, cols] with partition dim first."""
    return np.ascontiguousarray(
        mat_t.reshape(KC, 128, cols).transpose(1, 0, 2)).astype(bf16)


def make_in_maps(x, context, Wq, Wk, Wv, Wo):
    scale = np.float32(1.0 / np.sqrt(DH))
    # exp-bias skew tables per global head
    p = np.arange(128, dtype=np.int64)[:, None]
    t = np.arange(EBL, dtype=np.int64)[None, :]
    dist = np.abs(p + t - T0) // PERIOD          # [128, EBL]
    in_maps = []
    for c in range(8):
        b = c // 4
        h0 = (c % 4) * HL
        rows = slice(h0 * DH, (h0 + HL) * DH)
        xt = np.ascontiguousarray(
            x[b].T.reshape(KC, 128, T).transpose(1, 0, 2)).astype(bf16)
        ct = np.ascontiguousarray(
            context[b].T.reshape(KC, 128, T).transpose(1, 0, 2)).astype(bf16)
        wq = _to_chunked(np.ascontiguousarray((Wq[rows] * scale).T), 256)
        wk = _to_chunked(np.ascontiguousarray(Wk[rows].T), 256)
        wv = _to_chunked(np.ascontiguousarray(Wv[rows].T), 256)
        wo = np.ascontiguousarray(
            Wo[:, rows].T.reshape(2, 128, D).transpose(1, 0, 2)).astype(bf16)
        eb = np.empty((128, HL, EBL), dtype=bf16)
        for hl in range(HL):
            hs = 2.0 ** (-(h0 + hl + 1))
            eb[:, hl, :] = np.exp(-hs * dist).astype(bf16)
        in_maps.append({"xt": xt, "ct": ct, "wq": wq, "wk": wk, "wv": wv,
                        "wo": wo, "eb": np.ascontiguousarray(eb)})
    return in_maps


def kernel(x, context, Wq, Wk, Wv, Wo, bo, _collect=None):
    x = np.asarray(x, dtype=np.float32)
    context = np.asarray(context, dtype=np.float32)
    Wq = np.asarray(Wq, dtype=np.float32)
    Wk = np.asarray(Wk, dtype=np.float32)
    Wv = np.asarray(Wv, dtype=np.float32)
    Wo = np.asarray(Wo, dtype=np.float32)
    bo = np.asarray(bo, dtype=np.float32)

    nc = build_nc()
    in_maps = make_in_maps(x, context, Wq, Wk, Wv, Wo)
    res = run_bass_kernel_spmd(nc, in_maps, list(range(8)))
    if _collect is not None:
        _collect.append(res)

    out = np.empty((B, T, D), dtype=np.float32)
    for b in range(2):
        acc = res.results[4 * b]["y"].astype(np.float32)
        for c in range(4 * b + 1, 4 * b + 4):
            acc = acc + res.results[c]["y"].astype(np.float32)
        out[b] = acc + bo[None, :]
    return out


# revision 3
# speedup vs baseline: 1.0663x; 1.0663x over previous
"""Multi-head contextual biased attention on 8 Trainium2 NeuronCores.

Sharding: data-parallel over batch (B=2) x tensor-parallel over heads
(16 heads -> 4 per core). Each core computes Q/K/V projections for its
4 heads, streaming-softmax attention with the periodic ALiBi-style bias
applied as a precomputed multiplicative table (exp(bias) folded in after
exp(scores)), and the output projection. The host sums the 4 partial
output projections per batch element and adds the bias bo.

Device layout notes:
  - scores are computed transposed (S^T[j, i], context j on partitions) so
    the P@V contraction can run with V as the stationary operand; a ones
    column appended to V yields softmax denominators in the same matmul.
  - exp(bias*head_scale) depends only on (j - i), so it is stored as one
    skewed per-partition sequence eb[p, t] = g(p + t - T0) and addressed
    per tile with a step -1 access pattern; the multiply runs mostly on
    the DVE (bf16 2x mode) with ~1/5 of tiles on the Pool engine so the
    DVE total stays under the ACT exp stream.
  - the per-head softmax normalization runs engine-only (recip on the
    denominator row, partition_broadcast, multiply); the final multiply is
    emitted a few iterations into the NEXT head's loop so its wait on the
    Pool broadcast never head-of-line-blocks the DVE queue feeding PV.
  - phase 3 streams y out per [128,512] tile: PSUM -> bf16 cast
    (alternating DVE/ACT) -> ring DMA; the host accumulates partials.
"""

import numpy as np
import ml_dtypes
from contextlib import ExitStack

import concourse.bass as bass
import concourse.tile as tile
from concourse import bacc, mybir
from concourse.bass_utils import run_bass_kernel_spmd

bf16 = ml_dtypes.bfloat16
F32 = mybir.dt.float32
BF16 = mybir.dt.bfloat16
Exp = mybir.ActivationFunctionType.Exp

B, T, D = 2, 2048, 1024
NH, DH = 16, 64          # global heads, head dim
HL = 4                   # heads per core
KC = D // 128            # contraction chunks
PERIOD = 30
T0 = 2049                # odd skew origin (odd => step -1 APs stay 4B-aligned)
EBL = 3972               # skew table length


def _build_kernel(ctx, tc, y_d, xt_d, ct_d, wq_d, wk_d, wv_d, wo_d, eb_d):
    nc = tc.nc

    const = ctx.enter_context(tc.tile_pool(name="const", bufs=1))
    data = ctx.enter_context(tc.tile_pool(name="data", bufs=1))

    # DMA issue order matters: queues drain in issue order, so the q-path
    # inputs (wq, x) go first and bulk late-use loads (eb, wo) go last.
    wq_sb = const.tile([128, KC, 256], BF16)
    nc.sync.dma_start(wq_sb[:], wq_d[:])
    xt_sb = data.tile([128, KC, T], BF16)
    for k in range(KC):
        nc.sync.dma_start(xt_sb[:, k, :], xt_d[:, k, :])
    wk_sb = const.tile([128, KC, 256], BF16)
    nc.sync.dma_start(wk_sb[:], wk_d[:])
    wv_sb = const.tile([128, KC, 256], BF16)
    nc.sync.dma_start(wv_sb[:], wv_d[:])
    ct_sb = data.tile([128, KC, T], BF16)
    for k in range(KC):
        nc.sync.dma_start(ct_sb[:, k, :], ct_d[:, k, :])
    eb_sb = const.tile([128, HL, EBL], BF16)
    nc.sync.dma_start(eb_sb[:], eb_d[:])
    wo_sb = const.tile([128, 2, D], BF16)
    nc.sync.dma_start(wo_sb[:], wo_d[:])

    qT_sb = data.tile([128, 2, T], BF16)
    kT_sb = data.tile([128, 2, T], BF16)
    v_sb = data.tile([128, 16, HL, 65], BF16)
    o2_sb = data.tile([128, 2, T], BF16)
    nc.vector.memset(v_sb[:, :, :, 64:65], 1.0)

    # ---- Phase 1: projections ----
    with tc.tile_pool(name="pps", bufs=4, space="PSUM") as pps:
        # q^T[d, i] (scale folded into wq on host)
        for it in range(4):
            for m in range(2):
                ps = pps.tile([128, 512], F32, tag="mm", name=f"qps_{it}_{m}")
                for k in range(KC):
                    nc.tensor.matmul(ps[:], lhsT=wq_sb[:, k, m * 128:(m + 1) * 128],
                                     rhs=xt_sb[:, k, it * 512:(it + 1) * 512],
                                     start=(k == 0), stop=(k == KC - 1))
                nc.vector.tensor_copy(qT_sb[:, m, it * 512:(it + 1) * 512], ps[:])
        # k^T[d, j]
        for it in range(4):
            for m in range(2):
                ps = pps.tile([128, 512], F32, tag="mm", name=f"kps_{it}_{m}")
                for k in range(KC):
                    nc.tensor.matmul(ps[:], lhsT=wk_sb[:, k, m * 128:(m + 1) * 128],
                                     rhs=ct_sb[:, k, it * 512:(it + 1) * 512],
                                     start=(k == 0), stop=(k == KC - 1))
                nc.vector.tensor_copy(kT_sb[:, m, it * 512:(it + 1) * 512], ps[:])
        # v[j, d] in per-head stationary layout
        for jt in range(16):
            ps = pps.tile([128, 512], F32, tag="mm", name=f"vps_{jt}")
            for k in range(KC):
                nc.tensor.matmul(ps[:, 0:256], lhsT=ct_sb[:, k, jt * 128:(jt + 1) * 128],
                                 rhs=wv_sb[:, k, :], start=(k == 0), stop=(k == KC - 1))
            nc.vector.tensor_copy(
                v_sb[:, jt, :, 0:64],
                ps[:, 0:256].rearrange("p (h d) -> p h d", h=HL))

    # ---- Phase 2: attention per head ----
    # PV is software-pipelined one jt behind QK/exp/mult so the PE stream has
    # no dependency stall per jt.  Head order ends on an even head so the
    # final (exposed) normalization skips the odd-head staging DMA.
    with tc.tile_pool(name="sps", bufs=2, space="PSUM") as sps, \
         tc.tile_pool(name="pvs", bufs=1, space="PSUM") as pvs, \
         tc.tile_pool(name="pp", bufs=2) as pp, \
         tc.tile_pool(name="nrm", bufs=2) as nrm:
        pending = []
        for h in (0, 1, 3, 2):
            m = h // 2
            hp = (h % 2) * 64
            pv = pvs.tile([65, T], F32, tag="pv", name=f"pv_{h}")

            def emit_pv(jt, pt):
                for it in range(4):
                    nc.tensor.matmul(pv[:, it * 512:(it + 1) * 512],
                                     lhsT=v_sb[:, jt, h, :],
                                     rhs=pt[:, it * 512:(it + 1) * 512],
                                     start=(jt == 0), stop=(jt == 15))

            prev = None
            for jt in range(16):
                pt = pp.tile([128, T], BF16, tag="p", name=f"p_{h}_{jt}")
                idx0 = T0 + jt * 128
                for half in range(2):
                    sp = sps.tile([128, 1024], F32, tag="s", name=f"s_{h}_{jt}_{half}")
                    for it2 in range(2):
                        i0 = half * 1024 + it2 * 512
                        nc.tensor.matmul(sp[:, it2 * 512:(it2 + 1) * 512],
                                         lhsT=kT_sb[hp:hp + 64, m, jt * 128:(jt + 1) * 128],
                                         rhs=qT_sb[hp:hp + 64, m, i0:i0 + 512],
                                         start=True, stop=True)
                    hsl = slice(half * 1024, (half + 1) * 1024)
                    nc.scalar.activation(pt[:, hsl], sp[:], Exp)
                    ebs = eb_sb[:, h, idx0 - half * 1024:
                                idx0 - (half + 1) * 1024:-1]
                    # ~1/5 of bias multiplies go to the (slower) Pool engine so
                    # the DVE total stays below the ACT exp stream.
                    if (2 * jt + half) % 5 == 0:
                        nc.gpsimd.tensor_mul(pt[:, hsl], pt[:, hsl], ebs)
                    else:
                        nc.vector.tensor_mul(pt[:, hsl], pt[:, hsl], ebs)
                if prev is not None:
                    emit_pv(jt - 1, prev)
                prev = pt
                if jt == 4 and pending:
                    # deferred tail of the previous head's normalization: by
                    # now its Pool broadcast has long finished, so these DVE
                    # ops don't stall the queue.
                    for fn in pending:
                        fn()
                    pending = []
            emit_pv(15, prev)
            # fast psum release: copy pv -> sbuf, then normalize engine-only
            # (recip on the denominator row, broadcast, deferred multiply).
            pvf = nrm.tile([65, T], F32, tag="pvf", name=f"pvf_{h}")
            nc.vector.tensor_copy(pvf[:], pv[:])
            zr = nrm.tile([1, T], F32, tag="zr", name=f"zr_{h}")
            nc.vector.reciprocal(zr[:], pvf[64:65, :])
            zb = nrm.tile([64, T], F32, tag="zb", name=f"zb_{h}")
            nc.gpsimd.partition_broadcast(zb[:], zr[:], channels=64)

            def finish(h=h, m=m, pvf=pvf, zb=zb):
                if h % 2 == 0:
                    nc.vector.tensor_mul(o2_sb[0:64, m, :], pvf[0:64, :], zb[:])
                else:
                    otmp = nrm.tile([64, T], BF16, tag="otmp", name=f"otmp_{h}")
                    nc.vector.tensor_mul(otmp[:], pvf[0:64, :], zb[:])
                    nc.sync.dma_start(o2_sb[64:128, m, :], otmp[:])

            pending.append(finish)
        for fn in pending:
            fn()

    # ---- Phase 3: output projection (partial; host sums across head-groups) ----
    with tc.tile_pool(name="yps", bufs=3, space="PSUM") as yps, \
         tc.tile_pool(name="yo", bufs=8) as yo:
        for ic in range(16):
            for mt in range(2):
                ps = yps.tile([128, 512], F32, tag="y", name=f"yps_{ic}_{mt}")
                for m in range(2):
                    nc.tensor.matmul(ps[:], lhsT=o2_sb[:, m, ic * 128:(ic + 1) * 128],
                                     rhs=wo_sb[:, m, mt * 512:(mt + 1) * 512],
                                     start=(m == 0), stop=(m == 1))
                yt = yo.tile([128, 512], BF16, tag="yt", name=f"yt_{ic}_{mt}")
                # alternate cast engine so neither DVE nor ACT serializes the tail
                if mt == 0:
                    nc.vector.tensor_copy(yt[:], ps[:])
                else:
                    nc.scalar.copy(yt[:], ps[:])
                nc.sync.dma_start(y_d[ic * 128:(ic + 1) * 128, mt * 512:(mt + 1) * 512],
                                  yt[:])


_NC = None


def build_nc():
    global _NC
    if _NC is not None:
        return _NC
    nc = bacc.Bacc("TRN2", target_bir_lowering=False, debug=False, num_devices=8)
    xt_d = nc.dram_tensor("xt", [128, KC, T], BF16, kind="ExternalInput").ap()
    ct_d = nc.dram_tensor("ct", [128, KC, T], BF16, kind="ExternalInput").ap()
    wq_d = nc.dram_tensor("wq", [128, KC, 256], BF16, kind="ExternalInput").ap()
    wk_d = nc.dram_tensor("wk", [128, KC, 256], BF16, kind="ExternalInput").ap()
    wv_d = nc.dram_tensor("wv", [128, KC, 256], BF16, kind="ExternalInput").ap()
    wo_d = nc.dram_tensor("wo", [128, 2, D], BF16, kind="ExternalInput").ap()
    eb_d = nc.dram_tensor("eb", [128, HL, EBL], BF16, kind="ExternalInput").ap()
    y_d = nc.dram_tensor("y", [T, D], BF16, kind="ExternalOutput").ap()

    with tile.TileContext(nc) as tc, ExitStack() as ctx:
        _build_kernel(ctx, tc, y_d, xt_d, ct_d, wq_d, wk_d, wv_d, wo_d, eb_d)
    nc.compile()
    _NC = nc
    return nc


def _to_chunked(mat_t, cols):
    """[D, cols] -> [128, KC, cols] with partition dim first."""
    return np.ascontiguousarray(
        mat_t.reshape(KC, 128, cols).transpose(1, 0, 2)).astype(bf16)


def make_in_maps(x, context, Wq, Wk, Wv, Wo):
    scale = np.float32(1.0 / np.sqrt(DH))
    # exp-bias skew tables per global head
    p = np.arange(128, dtype=np.int64)[:, None]
    t = np.arange(EBL, dtype=np.int64)[None, :]
    dist = np.abs(p + t - T0) // PERIOD          # [128, EBL]
    in_maps = []
    for c in range(8):
        b = c // 4
        h0 = (c % 4) * HL
        rows = slice(h0 * DH, (h0 + HL) * DH)
        xt = np.ascontiguousarray(
            x[b].T.reshape(KC, 128, T).transpose(1, 0, 2)).astype(bf16)
        ct = np.ascontiguousarray(
            context[b].T.reshape(KC, 128, T).transpose(1, 0, 2)).astype(bf16)
        wq = _to_chunked(np.ascontiguousarray((Wq[rows] * scale).T), 256)
        wk = _to_chunked(np.ascontiguousarray(Wk[rows].T), 256)
        wv = _to_chunked(np.ascontiguousarray(Wv[rows].T), 256)
        wo = np.ascontiguousarray(
            Wo[:, rows].T.reshape(2, 128, D).transpose(1, 0, 2)).astype(bf16)
        eb = np.empty((128, HL, EBL), dtype=bf16)
        for hl in range(HL):
            hs = 2.0 ** (-(h0 + hl + 1))
            eb[:, hl, :] = np.exp(-hs * dist).astype(bf16)
        in_maps.append({"xt": xt, "ct": ct, "wq": wq, "wk": wk, "wv": wv,
                        "wo": wo, "eb": np.ascontiguousarray(eb)})
    return in_maps


def kernel(x, context, Wq, Wk, Wv, Wo, bo, _collect=None):
    x = np.asarray(x, dtype=np.float32)
    context = np.asarray(context, dtype=np.float32)
    Wq = np.asarray(Wq, dtype=np.float32)
    Wk = np.asarray(Wk, dtype=np.float32)
    Wv = np.asarray(Wv, dtype=np.float32)
    Wo = np.asarray(Wo, dtype=np.float32)
    bo = np.asarray(bo, dtype=np.float32)

    nc = build_nc()
    in_maps = make_in_maps(x, context, Wq, Wk, Wv, Wo)
    res = run_bass_kernel_spmd(nc, in_maps, list(range(8)))
    if _collect is not None:
        _collect.append(res)

    out = np.empty((B, T, D), dtype=np.float32)
    for b in range(2):
        acc = res.results[4 * b]["y"].astype(np.float32)
        for c in range(4 * b + 1, 4 * b + 4):
            acc = acc + res.results[c]["y"].astype(np.float32)
        out[b] = acc + bo[None, :]
    return out


# revision 4
# speedup vs baseline: 1.1821x; 1.1086x over previous
"""Multi-head contextual biased attention on 8 Trainium2 NeuronCores.

Sharding: data-parallel over batch (B=2) x tensor-parallel over heads
(16 heads -> 4 per core). Each core computes Q/K/V projections for its
4 heads, streaming-softmax attention with the periodic ALiBi-style bias
applied as a precomputed multiplicative table (exp(bias) folded in after
exp(scores)), and the output projection. The host sums the 4 partial
output projections per batch element and adds the bias bo.

Device layout notes:
  - scores are computed transposed (S^T[j, i], context j on partitions) so
    the P@V contraction can run with V as the stationary operand; a ones
    column appended to V yields softmax denominators in the same matmul.
  - exp(bias*head_scale) depends only on (j - i), so it is stored as one
    skewed per-partition sequence eb[p, t] = g(p + t - T0) and addressed
    per tile with a step -1 access pattern; the multiply runs mostly on
    the DVE (bf16 2x mode) with ~1/5 of tiles on the Pool engine so the
    DVE total stays under the ACT exp stream.
  - the per-head softmax normalization runs engine-only (recip on the
    denominator row, partition_broadcast, multiply); the final multiply is
    emitted a few iterations into the NEXT head's loop so its wait on the
    Pool broadcast never head-of-line-blocks the DVE queue feeding PV.
  - phase 3 streams y out per [128,512] tile: PSUM -> bf16 cast
    (alternating DVE/ACT) -> ring DMA; the host accumulates partials.
"""

import numpy as np
import ml_dtypes
from contextlib import ExitStack

import concourse.bass as bass
import concourse.tile as tile
from concourse import bacc, mybir
from concourse.bass_utils import run_bass_kernel_spmd

bf16 = ml_dtypes.bfloat16
F32 = mybir.dt.float32
BF16 = mybir.dt.bfloat16
Exp = mybir.ActivationFunctionType.Exp

B, T, D = 2, 2048, 1024
NH, DH = 16, 64          # global heads, head dim
HL = 4                   # heads per core
KC = D // 128            # contraction chunks
PERIOD = 30
T0 = 2049                # odd skew origin (odd => step -1 APs stay 4B-aligned)
EBL = 3972               # skew table length


def _build_kernel(ctx, tc, y_d, xt_d, ct_d, wq_d, wk_d, wv_d, wo_d, eb_d):
    nc = tc.nc

    const = ctx.enter_context(tc.tile_pool(name="const", bufs=1))
    data = ctx.enter_context(tc.tile_pool(name="data", bufs=1))

    # DMA issue order matters: queues drain in issue order, so the q-path
    # inputs (wq, x) go first and bulk late-use loads (eb, wo) go last.
    wq_sb = const.tile([128, KC, 256], BF16)
    nc.sync.dma_start(wq_sb[:], wq_d[:])
    xt_sb = data.tile([128, KC, T], BF16)
    for k in range(KC):
        nc.sync.dma_start(xt_sb[:, k, :], xt_d[:, k, :])
    wk_sb = const.tile([128, KC, 256], BF16)
    nc.sync.dma_start(wk_sb[:], wk_d[:])
    wv_sb = const.tile([128, KC, 256], BF16)
    nc.sync.dma_start(wv_sb[:], wv_d[:])
    ct_sb = data.tile([128, KC, T], BF16)
    for k in range(KC):
        nc.sync.dma_start(ct_sb[:, k, :], ct_d[:, k, :])
    eb_sb = const.tile([128, HL, EBL], BF16)
    nc.sync.dma_start(eb_sb[:], eb_d[:])
    wo_sb = const.tile([128, 2, D], BF16)
    nc.sync.dma_start(wo_sb[:], wo_d[:])

    qT_sb = data.tile([128, 2, T], BF16)
    kT_sb = data.tile([128, 2, T], BF16)
    v_sb = data.tile([128, 16, HL, 65], BF16)
    o2_sb = data.tile([128, 2, T], BF16)
    nc.vector.memset(v_sb[:, :, :, 64:65], 1.0)

    # ---- Phase 1: projections ----
    with tc.tile_pool(name="pps", bufs=4, space="PSUM") as pps:
        # q^T[d, i] (scale folded into wq on host)
        for it in range(4):
            for m in range(2):
                ps = pps.tile([128, 512], F32, tag="mm", name=f"qps_{it}_{m}")
                for k in range(KC):
                    nc.tensor.matmul(ps[:], lhsT=wq_sb[:, k, m * 128:(m + 1) * 128],
                                     rhs=xt_sb[:, k, it * 512:(it + 1) * 512],
                                     start=(k == 0), stop=(k == KC - 1))
                nc.vector.tensor_copy(qT_sb[:, m, it * 512:(it + 1) * 512], ps[:])
        # k^T[d, j]
        for it in range(4):
            for m in range(2):
                ps = pps.tile([128, 512], F32, tag="mm", name=f"kps_{it}_{m}")
                for k in range(KC):
                    nc.tensor.matmul(ps[:], lhsT=wk_sb[:, k, m * 128:(m + 1) * 128],
                                     rhs=ct_sb[:, k, it * 512:(it + 1) * 512],
                                     start=(k == 0), stop=(k == KC - 1))
                nc.vector.tensor_copy(kT_sb[:, m, it * 512:(it + 1) * 512], ps[:])
        # v[j, d] in per-head stationary layout
        for jt in range(16):
            ps = pps.tile([128, 512], F32, tag="mm", name=f"vps_{jt}")
            for k in range(KC):
                nc.tensor.matmul(ps[:, 0:256], lhsT=ct_sb[:, k, jt * 128:(jt + 1) * 128],
                                 rhs=wv_sb[:, k, :], start=(k == 0), stop=(k == KC - 1))
            nc.vector.tensor_copy(
                v_sb[:, jt, :, 0:64],
                ps[:, 0:256].rearrange("p (h d) -> p h d", h=HL))

    # ---- Phase 2: attention per head ----
    # PV is software-pipelined one jt behind QK/exp/mult so the PE stream has
    # no dependency stall per jt.  Head order ends on an even head so the
    # final (exposed) normalization skips the odd-head staging DMA.
    with tc.tile_pool(name="sps", bufs=2, space="PSUM") as sps, \
         tc.tile_pool(name="pvs", bufs=1, space="PSUM") as pvs, \
         tc.tile_pool(name="pp", bufs=2) as pp, \
         tc.tile_pool(name="nrm", bufs=2) as nrm:
        pending = []
        for h in (0, 1, 3, 2):
            m = h // 2
            hp = (h % 2) * 64
            pv = pvs.tile([65, T], F32, tag="pv", name=f"pv_{h}")

            def emit_pv(jt, pt):
                for it in range(4):
                    nc.tensor.matmul(pv[:, it * 512:(it + 1) * 512],
                                     lhsT=v_sb[:, jt, h, :],
                                     rhs=pt[:, it * 512:(it + 1) * 512],
                                     start=(jt == 0), stop=(jt == 15))

            prev = None
            for jt in range(16):
                pt = pp.tile([128, T], BF16, tag="p", name=f"p_{h}_{jt}")
                idx0 = T0 + jt * 128
                for half in range(2):
                    sp = sps.tile([128, 1024], F32, tag="s", name=f"s_{h}_{jt}_{half}")
                    for it2 in range(2):
                        i0 = half * 1024 + it2 * 512
                        nc.tensor.matmul(sp[:, it2 * 512:(it2 + 1) * 512],
                                         lhsT=kT_sb[hp:hp + 64, m, jt * 128:(jt + 1) * 128],
                                         rhs=qT_sb[hp:hp + 64, m, i0:i0 + 512],
                                         start=True, stop=True)
                    hsl = slice(half * 1024, (half + 1) * 1024)
                    nc.scalar.activation(pt[:, hsl], sp[:], Exp)
                    ebs = eb_sb[:, h, idx0 - half * 1024:
                                idx0 - (half + 1) * 1024:-1]
                    # ~1/5 of bias multiplies go to the (slower) Pool engine so
                    # the DVE total stays below the ACT exp stream.
                    if (2 * jt + half) % 5 == 0:
                        nc.gpsimd.tensor_mul(pt[:, hsl], pt[:, hsl], ebs)
                    else:
                        nc.vector.tensor_mul(pt[:, hsl], pt[:, hsl], ebs)
                if prev is not None:
                    emit_pv(jt - 1, prev)
                prev = pt
                # deferred stages of the previous head's normalization: each
                # stage's producer finished several slots ago, so these never
                # head-of-line-block an engine queue on an unmet dependency.
                for trig, fn in pending:
                    if trig == jt:
                        fn()
            emit_pv(15, prev)
            pending = [(t, f) for (t, f) in pending if t > jt]
            # fast psum release: copy pv -> sbuf; the denominator row is
            # DMA-reshaped to [128, 16] so the reciprocal runs across all
            # partitions (a [1, T] reciprocal would serialize on one lane).
            pvf = nrm.tile([65, T], F32, tag="pvf", name=f"pvf_{h}")
            nc.vector.tensor_copy(pvf[:], pv[:])
            rsq = nrm.tile([128, 16], F32, tag="rsq", name=f"rsq_{h}")
            nc.sync.dma_start(rsq[:], pvf[64:65, :])
            rsr = nrm.tile([128, 16], F32, tag="rsr", name=f"rsr_{h}")
            rsf = nrm.tile([1, T], F32, tag="rsf", name=f"rsf_{h}")
            rsb = nrm.tile([64, T], F32, tag="rsb", name=f"rsb_{h}")

            def st_recip(rsr=rsr, rsq=rsq):
                nc.vector.reciprocal(rsr[:], rsq[:])

            def st_rsf(rsf=rsf, rsr=rsr):
                nc.sync.dma_start(rsf[:], rsr[:])

            def st_bcast(rsb=rsb, rsf=rsf):
                nc.gpsimd.partition_broadcast(rsb[:], rsf[:], channels=64)

            def st_mul(h=h, m=m, pvf=pvf, rsb=rsb):
                if h % 2 == 0:
                    nc.vector.tensor_mul(o2_sb[0:64, m, :], pvf[0:64, :], rsb[:])
                else:
                    otmp = nrm.tile([64, T], BF16, tag="otmp", name=f"otmp_{h}")
                    nc.vector.tensor_mul(otmp[:], pvf[0:64, :], rsb[:])
                    nc.sync.dma_start(o2_sb[64:128, m, :], otmp[:])

            pending += [(3, st_recip), (6, st_rsf), (9, st_bcast), (12, st_mul)]
        for _, fn in pending:
            fn()

    # ---- Phase 3: output projection (partial; host sums across head-groups) ----
    with tc.tile_pool(name="yps", bufs=3, space="PSUM") as yps, \
         tc.tile_pool(name="yo", bufs=8) as yo:
        for ic in range(16):
            for mt in range(2):
                ps = yps.tile([128, 512], F32, tag="y", name=f"yps_{ic}_{mt}")
                for m in range(2):
                    nc.tensor.matmul(ps[:], lhsT=o2_sb[:, m, ic * 128:(ic + 1) * 128],
                                     rhs=wo_sb[:, m, mt * 512:(mt + 1) * 512],
                                     start=(m == 0), stop=(m == 1))
                yt = yo.tile([128, 512], BF16, tag="yt", name=f"yt_{ic}_{mt}")
                # alternate cast engine so neither DVE nor ACT serializes the tail
                if mt == 0:
                    nc.vector.tensor_copy(yt[:], ps[:])
                else:
                    nc.scalar.copy(yt[:], ps[:])
                nc.sync.dma_start(y_d[ic * 128:(ic + 1) * 128, mt * 512:(mt + 1) * 512],
                                  yt[:])


_NC = None


def build_nc():
    global _NC
    if _NC is not None:
        return _NC
    nc = bacc.Bacc("TRN2", target_bir_lowering=False, debug=False, num_devices=8)
    xt_d = nc.dram_tensor("xt", [128, KC, T], BF16, kind="ExternalInput").ap()
    ct_d = nc.dram_tensor("ct", [128, KC, T], BF16, kind="ExternalInput").ap()
    wq_d = nc.dram_tensor("wq", [128, KC, 256], BF16, kind="ExternalInput").ap()
    wk_d = nc.dram_tensor("wk", [128, KC, 256], BF16, kind="ExternalInput").ap()
    wv_d = nc.dram_tensor("wv", [128, KC, 256], BF16, kind="ExternalInput").ap()
    wo_d = nc.dram_tensor("wo", [128, 2, D], BF16, kind="ExternalInput").ap()
    eb_d = nc.dram_tensor("eb", [128, HL, EBL], BF16, kind="ExternalInput").ap()
    y_d = nc.dram_tensor("y", [T, D], BF16, kind="ExternalOutput").ap()

    with tile.TileContext(nc) as tc, ExitStack() as ctx:
        _build_kernel(ctx, tc, y_d, xt_d, ct_d, wq_d, wk_d, wv_d, wo_d, eb_d)
    nc.compile()
    _NC = nc
    return nc


def _to_chunked(mat_t, cols):
    """[D, cols] -> [128, KC, cols] with partition dim first."""
    return np.ascontiguousarray(
        mat_t.reshape(KC, 128, cols).transpose(1, 0, 2)).astype(bf16)


def make_in_maps(x, context, Wq, Wk, Wv, Wo):
    scale = np.float32(1.0 / np.sqrt(DH))
    # exp-bias skew tables per global head
    p = np.arange(128, dtype=np.int64)[:, None]
    t = np.arange(EBL, dtype=np.int64)[None, :]
    dist = np.abs(p + t - T0) // PERIOD          # [128, EBL]
    in_maps = []
    for c in range(8):
        b = c // 4
        h0 = (c % 4) * HL
        rows = slice(h0 * DH, (h0 + HL) * DH)
        xt = np.ascontiguousarray(
            x[b].T.reshape(KC, 128, T).transpose(1, 0, 2)).astype(bf16)
        ct = np.ascontiguousarray(
            context[b].T.reshape(KC, 128, T).transpose(1, 0, 2)).astype(bf16)
        wq = _to_chunked(np.ascontiguousarray((Wq[rows] * scale).T), 256)
        wk = _to_chunked(np.ascontiguousarray(Wk[rows].T), 256)
        wv = _to_chunked(np.ascontiguousarray(Wv[rows].T), 256)
        wo = np.ascontiguousarray(
            Wo[:, rows].T.reshape(2, 128, D).transpose(1, 0, 2)).astype(bf16)
        eb = np.empty((128, HL, EBL), dtype=bf16)
        for hl in range(HL):
            hs = 2.0 ** (-(h0 + hl + 1))
            eb[:, hl, :] = np.exp(-hs * dist).astype(bf16)
        in_maps.append({"xt": xt, "ct": ct, "wq": wq, "wk": wk, "wv": wv,
                        "wo": wo, "eb": np.ascontiguousarray(eb)})
    return in_maps


def kernel(x, context, Wq, Wk, Wv, Wo, bo, _collect=None):
    x = np.asarray(x, dtype=np.float32)
    context = np.asarray(context, dtype=np.float32)
    Wq = np.asarray(Wq, dtype=np.float32)
    Wk = np.asarray(Wk, dtype=np.float32)
    Wv = np.asarray(Wv, dtype=np.float32)
    Wo = np.asarray(Wo, dtype=np.float32)
    bo = np.asarray(bo, dtype=np.float32)

    nc = build_nc()
    in_maps = make_in_maps(x, context, Wq, Wk, Wv, Wo)
    res = run_bass_kernel_spmd(nc, in_maps, list(range(8)))
    if _collect is not None:
        _collect.append(res)

    out = np.empty((B, T, D), dtype=np.float32)
    for b in range(2):
        acc = res.results[4 * b]["y"].astype(np.float32)
        for c in range(4 * b + 1, 4 * b + 4):
            acc = acc + res.results[c]["y"].astype(np.float32)
        out[b] = acc + bo[None, :]
    return out
